# revision 14
# baseline (speedup 1.0000x reference)
"""Additive attention (Bahdanau) Trainium2 kernel, SPMD across 8 NeuronCores.

Reference computation (per batch b):
    q = Q[b] @ Wq                 [NQ, H]
    k = K[b] @ Wk                 [NK, H]
    scores[i, j] = sum_h Wv[h] * tanh(q[i, h] + k[j, h])
    attn = softmax(mask(scores))  (keys >= valid_len[b] masked to -1e6)
    out[b] = attn @ V[b]

Sharding: core c handles queries [c*QG, (c+1)*QG) of EVERY batch (QG =
NQ/8).  Each batch's key range is truncated to its valid_len at
graph-build time (valid_lens are host-visible), so no masking is ever
needed: keys beyond valid_len contribute exactly 0 to the reference
softmax (exp(-1e6) underflows to 0), so truncation is exact.  Softmax is
computed without max-subtraction: |scores| <= sum|Wv| ~ 10, safely inside
f32/bf16 exp range, and the reference ratio is identical.

Engine mapping (per core, per batch-group g of QG queries x vl_g keys):
  PE   : qT/kT projections; per-(query, key-block) score matmuls
         (lhsT = tanh tile [H, keys], rhs = Wv [H, 1]) which produce the
         scores TRANSPOSED [keys, query] - exactly the attn @ V layout;
         softmax-denominator ones-matmuls; attn @ V matmuls.
  DVE  : per-query broadcast adds q[:,qi] + kT (tensor_scalar, bf16 4x);
         projection PSUM->SBUF copies; reciprocal; final 1/Z scaling.
  ACT  : big-tile tanh (bf16); exp of the transposed scores.
  SYNC : all DMA (HWDGE).
All reductions accumulate in f32 PSUM; bf16 only on storage/stream paths.
"""

import math

import numpy as np
import ml_dtypes

import concourse.bass as bass
import concourse.mybir as mybir
from concourse.bass_utils import run_bass_kernel_spmd

BF16 = mybir.dt.bfloat16
F32 = mybir.dt.float32
AF = mybir.ActivationFunctionType

N_CORES = 8


def build_graph(vls, B=4, H=128, DQ=512, DK=512, DV=512, QG=64, QC=16, iters=1,
                bench=(), tanh_f32=False):
    """Build the per-core bass graph. vls: per-batch valid lens (python ints).

    iters > 1 unrolls the whole group pipeline iters times back-to-back in
    one NEFF (same data), for marginal-cost timing immune to dispatch
    overhead.  Group indices gg run over iters*B; batch identity is gg %% B.
    """
    assert H == 128 and DQ % 128 == 0 and DK % 128 == 0
    NCH = QG // QC
    assert NCH * QC == QG and NCH % 2 == 0
    GG = iters * B
    W = [int(v + (v & 1)) for v in vls]          # even widths (DVE 4x mode)
    offs = [0]
    for w in W:
        offs.append(offs[-1] + w)
    Wsum = offs[-1]
    Wmax = max(W)
    nb = [max(1, math.ceil(v / 128)) for v in vls]  # key-blocks per group
    nbmax = max(nb)
    vco = [0]
    for n in nb:
        vco.append(vco[-1] + n)
    NVC = vco[-1]
    nDQ, nDK = DQ // 128, DK // 128

    nc = bass.Bass()

    qT_e = nc.declare_dram_parameter("qT", [128, nDQ, B, QG], BF16, isOutput=False)
    kT_e = nc.declare_dram_parameter("kT", [128, nDK, Wsum], BF16, isOutput=False)
    v_e = nc.declare_dram_parameter("v", [128, NVC, DV], BF16, isOutput=False)
    wq_e = nc.declare_dram_parameter("wq", [128, nDQ, H], BF16, isOutput=False)
    wk_e = nc.declare_dram_parameter("wk", [128, nDK, H], BF16, isOutput=False)
    wv_e = nc.declare_dram_parameter("wv", [H, 1], BF16, isOutput=False)
    ones_e = nc.declare_dram_parameter("ones", [128, 1], BF16, isOutput=False)
    out_e = nc.declare_dram_parameter("out", [B, QG, DV], F32, isOutput=True)

    # ---- pass A: enumerate semaphore-inc orders per engine -----------------
    class S:
        def __init__(self):
            self.n = 0
            self.idx = {}

        def inc(self, tag):
            self.n += 1
            if tag is not None:
                self.idx[tag] = self.n
            return self.n

    dma, pe, act, dve = S(), S(), S(), S()

    LOADS = ["wq", "wk", "qT", "kT", "wv", "ones", "v"]

    # pe order: 8 proj groups (q0,k0,q1,k1,...), then per-group score-matmul
    # chunks, with z(g-1)+vm(g-1) interleaved right after chunk (g, 0).
    for i in range(2 * B):
        pe.inc(("proj", i))
    for gg in range(GG):
        for c in range(NCH):
            pe.inc(("m", gg, c))
            if c == 0 and gg >= 1:
                pe.inc(("z", gg - 1))
                pe.inc(("vm", gg - 1))
    pe.inc(("z", GG - 1))
    pe.inc(("vm", GG - 1))

    # act order: tanh chunks; exp(g) deferred until after tanh(g+1, 0)
    act_order = []
    for gg in range(GG):
        for c in range(NCH):
            act_order.append(("t", gg, c))
            if gg >= 1 and c == 0:
                act_order.append(("e", gg - 1))
    act_order.append(("e", GG - 1))
    for tag in act_order:
        act.inc(tag)

    # dve order: memset, proj copies, adds with epilogue(g-1) interleaved
    dve.inc(("ms",))
    dve.inc(("msc0",))
    dve.inc(("msc1",))
    for i in range(2 * B):
        dve.inc(("pc", i))
    for gg in range(GG):
        for c in range(NCH):
            dve.inc(("a", gg, c))
            if c == 1 and gg >= 1:
                dve.inc(("r", gg - 1))
            if c == 2 and gg >= 1:
                dve.inc(("o", gg - 1))
    dve.inc(("r", GG - 1))
    dve.inc(("o", GG - 1))

    def chunk_gc(tau):
        # global chunk index -> (gg, c)
        return tau // NCH, tau % NCH

    # ---- allocate memory + emit ------------------------------------------
    from contextlib import ExitStack

    es = ExitStack()
    with es:
        wq_sb = es.enter_context(nc.sbuf_tensor([128, nDQ, H], BF16))
        wk_sb = es.enter_context(nc.sbuf_tensor([128, nDK, H], BF16))
        qT_sb = es.enter_context(nc.sbuf_tensor([128, nDQ, B, QG], BF16))
        kT_sb = es.enter_context(nc.sbuf_tensor([128, nDK, Wsum], BF16))
        v_sb = es.enter_context(nc.sbuf_tensor([128, NVC, DV], BF16))
        wv_sb = es.enter_context(nc.sbuf_tensor([128, 1], BF16))
        ones_sb = es.enter_context(nc.sbuf_tensor([128, 1], BF16))
        qTp_sb = es.enter_context(nc.sbuf_tensor([128, B, QG], F32))
        kTp_sb = es.enter_context(nc.sbuf_tensor([128, Wsum], BF16))
        sum_sb = es.enter_context(nc.sbuf_tensor([128, 2, QC * Wmax], BF16))
        tanh_sb = es.enter_context(
            nc.sbuf_tensor([128, 2, QC * Wmax], F32 if tanh_f32 else BF16)
        )
        exp_sb = es.enter_context(nc.sbuf_tensor([128, 2, nbmax, QG], BF16))
        recip_sb = es.enter_context(nc.sbuf_tensor([QG, B], F32))
        out_sb = es.enter_context(nc.sbuf_tensor([QG, 2, DV], F32))
        scratch = es.enter_context(nc.sbuf_tensor([1, 8], F32))
        scratch2 = es.enter_context(nc.sbuf_tensor([1, 8], F32))

        pp = [
            es.enter_context(nc.psum_tensor(f"pp{i}", [128, 512], F32))
            for i in range(2)
        ]
        sc = [
            es.enter_context(nc.psum_tensor(f"sc{i}", [128, nbmax, QG], F32))
            for i in range(2)
        ]
        op = [
            es.enter_context(nc.psum_tensor(f"op{i}", [QG, DV], F32))
            for i in range(2)
        ]
        z_ps = es.enter_context(nc.psum_tensor("z_ps", [QG, B], F32))

        ld_sem = {
            name: es.enter_context(nc.semaphore(f"ld_{name}")) for name in LOADS
        }
        ost_sem = [
            es.enter_context(nc.semaphore(f"ost{i}")) for i in range(2)
        ]
        pe_sem = es.enter_context(nc.semaphore("pe_sem"))
        act_sem = es.enter_context(nc.semaphore("act_sem"))
        dve_sem = es.enter_context(nc.semaphore("dve_sem"))
        block = es.enter_context(nc.Block())

        @block.sync
        def _(sy):
            sy.dma_start(out=wq_sb[:], in_=wq_e[:]).then_inc(ld_sem["wq"], 16)
            sy.dma_start(out=wk_sb[:], in_=wk_e[:]).then_inc(ld_sem["wk"], 16)
            sy.dma_start(out=qT_sb[:], in_=qT_e[:]).then_inc(ld_sem["qT"], 16)
            sy.dma_start(out=kT_sb[:], in_=kT_e[:]).then_inc(ld_sem["kT"], 16)
            sy.dma_start(out=wv_sb[:], in_=wv_e[:]).then_inc(ld_sem["wv"], 16)
            sy.dma_start(out=ones_sb[:], in_=ones_e[:]).then_inc(ld_sem["ones"], 16)
            sy.dma_start(out=v_sb[:], in_=v_e[:]).then_inc(ld_sem["v"], 16)
            for gg in range(GG):
                sy.wait_ge(dve_sem, dve.idx[("o", gg)])
                sy.dma_start(
                    out=out_e[gg % B], in_=out_sb[0:QG, gg % 2, :]
                ).then_inc(ost_sem[gg % 2], 16)

        @block.tensor
        def _(pe_eng):
            def proj(i):
                is_q, g = (i % 2 == 0), i // 2
                if i == 0:
                    pe_eng.wait_ge(ld_sem["wq"], 16)
                    pe_eng.wait_ge(ld_sem["qT"], 16)
                if i == 1:
                    pe_eng.wait_ge(ld_sem["wk"], 16)
                    pe_eng.wait_ge(ld_sem["kT"], 16)
                if i >= 2:
                    pe_eng.wait_ge(dve_sem, dve.idx[("pc", i - 2)])
                nch = nDQ if is_q else nDK
                for c in range(nch):
                    if is_q:
                        o, l, r = (
                            pp[i % 2][0:128, 0:QG],
                            wq_sb[:, c, :],
                            qT_sb[:, c, g, :],
                        )
                    else:
                        o, l, r = (
                            pp[i % 2][0:128, 0 : W[g]],
                            wk_sb[:, c, :],
                            kT_sb[:, c, offs[g] : offs[g] + W[g]],
                        )
                    mm = pe_eng.matmul(o, l, r, start=(c == 0), stop=(c == nch - 1))
                mm.then_inc(pe_sem, 1)

            for i in range(2 * B):
                proj(i)

            pe_eng.wait_ge(ld_sem["wv"], 16)

            def mm_chunk(gg, c):
                g = gg % B
                pe_eng.wait_ge(act_sem, act.idx[("t", gg, c)])
                for qq in range(1 if "mm" in bench else QC):
                    qi = c * QC + qq
                    for b in range(nb[g]):
                        sz = min(128, vls[g] - 128 * b)
                        mm = pe_eng.matmul(
                            sc[gg % 2][0:sz, b, qi : qi + 1],
                            tanh_sb[
                                :, c % 2, qq * W[g] + 128 * b : qq * W[g] + 128 * b + sz
                            ],
                            wv_sb[:, 0:1],
                            start=True,
                            stop=True,
                        )
                mm.then_inc(pe_sem, 1)

            def zmm(gg):
                g = gg % B
                pe_eng.wait_ge(act_sem, act.idx[("e", gg)])
                if gg == 0:
                    pe_eng.wait_ge(ld_sem["ones"], 16)
                for b in range(nb[g]):
                    sz = min(128, vls[g] - 128 * b)
                    mm = pe_eng.matmul(
                        z_ps[0:QG, g : g + 1],
                        exp_sb[0:sz, gg % 2, b, :],
                        ones_sb[0:sz, :],
                        start=(b == 0),
                        stop=(b == nb[g] - 1),
                    )
                mm.then_inc(pe_sem, 1)

            def vmm(gg):
                g = gg % B
                if gg == 0:
                    pe_eng.wait_ge(ld_sem["v"], 16)
                for b in range(nb[g]):
                    sz = min(128, vls[g] - 128 * b)
                    mm = pe_eng.matmul(
                        op[gg % 2][0:QG, 0:DV],
                        exp_sb[0:sz, gg % 2, b, :],
                        v_sb[0:sz, vco[g] + b, :],
                        start=(b == 0),
                        stop=(b == nb[g] - 1),
                    )
                mm.then_inc(pe_sem, 1)

            for gg in range(GG):
                for c in range(NCH):
                    mm_chunk(gg, c)
                    if c == 0 and gg >= 1:
                        zmm(gg - 1)
                        vmm(gg - 1)
            zmm(GG - 1)
            vmm(GG - 1)

        @block.scalar
        def _(sa):
            sa.wait_ge(dve_sem, dve.idx[("ms",)])
            sa.activation(scratch2[0:1, 0:2], scratch[0:1, 0:2], AF.Tanh)

            def tanh_op(gg, c):
                g = gg % B
                tau = gg * NCH + c
                sa.wait_ge(dve_sem, dve.idx[("a", gg, c)])
                if tau >= 2:
                    g2, c2 = chunk_gc(tau - 2)
                    sa.wait_ge(pe_sem, pe.idx[("m", g2, c2)])
                tw = 64 if "tanh" in bench else QC * W[g]
                sa.activation(
                    tanh_sb[:, c % 2, 0:tw],
                    sum_sb[:, c % 2, 0:tw],
                    AF.Tanh,
                ).then_inc(act_sem, 1)

            def exp_op(gg):
                g = gg % B
                sa.wait_ge(pe_sem, pe.idx[("m", gg, NCH - 1)])
                sa.activation(
                    exp_sb[0:128, gg % 2, 0 : nb[g], :],
                    sc[gg % 2][0:128, 0 : nb[g], :],
                    AF.Exp,
                ).then_inc(act_sem, 1)

            for tag in act_order:
                if tag[0] == "t":
                    tanh_op(tag[1], tag[2])
                else:
                    exp_op(tag[1])

        @block.vector
        def _(ve):
            ve.memset(scratch[0:1, 0:8], 0.0).then_inc(dve_sem, 1)
            ve.memset(sc[0][:], 0.0).then_inc(dve_sem, 1)
            ve.memset(sc[1][:], 0.0).then_inc(dve_sem, 1)

            def proj_copy(i):
                is_q, g = (i % 2 == 0), i // 2
                ve.wait_ge(pe_sem, pe.idx[("proj", i)])
                if is_q:
                    cp = ve.tensor_copy(qTp_sb[:, g, :], pp[i % 2][0:128, 0:QG])
                else:
                    cp = ve.tensor_copy(
                        kTp_sb[:, offs[g] : offs[g] + W[g]],
                        pp[i % 2][0:128, 0 : W[g]],
                    )
                cp.then_inc(dve_sem, 1)

            for i in range(2 * B):
                proj_copy(i)

            def adds(gg, c):
                g = gg % B
                tau = gg * NCH + c
                if c == 0 and gg < B:
                    # scalar-ptr operands are prefetched at issue: wait for our
                    # own q-projection copy's sem inc before reading qTp scalars
                    ve.wait_ge(dve_sem, dve.idx[("pc", 2 * g)])
                if tau >= 2:
                    g2, c2 = chunk_gc(tau - 2)
                    ve.wait_ge(act_sem, act.idx[("t", g2, c2)])
                for qq in range(1 if "add" in bench else QC):
                    qi = c * QC + qq
                    a = ve.tensor_scalar_add(
                        sum_sb[:, c % 2, qq * W[g] : (qq + 1) * W[g]],
                        kTp_sb[:, offs[g] : offs[g] + W[g]],
                        qTp_sb[:, g, qi : qi + 1],
                    )
                a.then_inc(dve_sem, 1)

            def ep_recip(gg):
                g = gg % B
                ve.wait_ge(pe_sem, pe.idx[("z", gg)])
                ve.reciprocal(
                    recip_sb[0:QG, g : g + 1], z_ps[0:QG, g : g + 1]
                ).then_inc(dve_sem, 1)

            def ep_oscale(gg):
                g = gg % B
                ve.wait_ge(dve_sem, dve.idx[("r", gg)])  # recip scalar-ptr hazard
                ve.wait_ge(pe_sem, pe.idx[("vm", gg)])
                if gg >= 2:
                    ve.wait_ge(ost_sem[gg % 2], 16 * (gg // 2))
                ve.tensor_scalar_mul(
                    out_sb[0:QG, gg % 2, :],
                    op[gg % 2][0:QG, 0:DV],
                    recip_sb[0:QG, g : g + 1],
                ).then_inc(dve_sem, 1)

            for gg in range(GG):
                for c in range(NCH):
                    adds(gg, c)
                    if c == 1 and gg >= 1:
                        ep_recip(gg - 1)
                    if c == 2 and gg >= 1:
                        ep_oscale(gg - 1)
            ep_recip(GG - 1)
            ep_oscale(GG - 1)

    return nc


def _host_prep(queries, keys, values, Wq, Wk, Wv, valid_lens,
               B, H, DQ, DK, DV, QG, QC):
    bf = ml_dtypes.bfloat16
    vls = [int(v) for v in np.asarray(valid_lens)]
    W = [int(v + (v & 1)) for v in vls]
    offs = [0]
    for w in W:
        offs.append(offs[-1] + w)
    Wsum = offs[-1]
    nb = [max(1, math.ceil(v / 128)) for v in vls]
    vco = [0]
    for n in nb:
        vco.append(vco[-1] + n)
    NVC = vco[-1]

    nDQ, nDK = DQ // 128, DK // 128
    kT = np.zeros((DK, Wsum), np.float32)
    for b in range(B):
        kb = np.asarray(keys[b][: vls[b]]).T  # [DK, vl]
        kT[:, offs[b] : offs[b] + vls[b]] = kb
        if W[b] > vls[b]:
            kT[:, offs[b] + vls[b]] = kb[:, -1]
    kT = kT.reshape(nDK, 128, Wsum).transpose(1, 0, 2)  # [128, nDK, Wsum]
    v = np.zeros((128 * NVC, DV), np.float32)
    for b in range(B):
        v[128 * vco[b] : 128 * vco[b] + vls[b]] = values[b][: vls[b]]
    v = v.reshape(NVC, 128, DV).transpose(1, 0, 2)  # [128, NVC, DV]
    wq = np.asarray(Wq).reshape(nDQ, 128, H).transpose(1, 0, 2)
    wk = np.asarray(Wk).reshape(nDK, 128, H).transpose(1, 0, 2)
    # [128, nDQ, B, NQ]
    qT_full = np.asarray(queries).transpose(0, 2, 1).reshape(B, nDQ, 128, -1)
    qT_full = qT_full.transpose(2, 1, 0, 3)

    common = {
        "kT": np.ascontiguousarray(kT).astype(bf),
        "v": np.ascontiguousarray(v).astype(bf),
        "wq": np.ascontiguousarray(wq).astype(bf),
        "wk": np.ascontiguousarray(wk).astype(bf),
        "wv": np.ascontiguousarray(np.asarray(Wv).reshape(H, 1)).astype(bf),
        "ones": np.ones((128, 1), dtype=bf),
    }
    in_maps = []
    for c in range(N_CORES):
        m = dict(common)
        m["qT"] = np.ascontiguousarray(
            qT_full[:, :, :, c * QG : (c + 1) * QG]
        ).astype(bf)
        in_maps.append(m)
    return vls, in_maps


def kernel(queries, keys, values, Wq, Wk, Wv, valid_lens):
    B, NQ, DQ = queries.shape
    _, NK, DK = keys.shape
    DV = values.shape[2]
    H = Wq.shape[1]
    QG = NQ // N_CORES
    QC = 16 if QG % 16 == 0 else (8 if QG % 8 == 0 else QG)

    vls, in_maps = _host_prep(
        queries, keys, values, Wq, Wk, Wv, valid_lens, B, H, DQ, DK, DV, QG, QC
    )
    nc = build_graph(vls, B=B, H=H, DQ=DQ, DK=DK, DV=DV, QG=QG, QC=QC)
    r = run_bass_kernel_spmd(nc, in_maps, core_ids=list(range(N_CORES)))
    out = np.empty((B, NQ, DV), np.float32)
    for c in range(N_CORES):
        out[:, c * QG : (c + 1) * QG, :] = r.results[c]["out"]
    return out
